# revision 9
# baseline (speedup 1.0000x reference)
"""Trainium2 Bass kernel for nn_DUMFNet (scale/channel-attention bridge).

Pure data parallel over batch: 32 samples -> 8 NeuronCores x 4 samples.
Per core, for each scale i (C_i channels, HxW spatial), with t in SBUF as
[partition = h (sample-packed for small scales), free = (c, w)]:
  a    = [sum_c t / C, max_c t]           (DVE reduces over a strided view)
  z    = dilated 7x7 conv(a)              (TensorE banded matmuls, bf16)
  sa   = sigmoid(z + sp_b)                (ScalarE, from PSUM)
  u_c  = (1+sa) * t[:,c,:]                (DVE scalar_tensor_tensor,
                                           fused accum_out row-sums)
  pooled = mean_hw(u)                     (small TensorE matmuls on accums)
  att  = conv1d_k3(concat_c pooled)       (TensorE banded matmul)
  catt = sigmoid(att @ w_i^T + b_i)       (TensorE + ScalarE)
  out  = ((1+catt)*(1+sa) - 1) * t        (two fused DVE passes w/ broadcasts)
which equals the reference out = catt*u + sa*t exactly, since u = (1+sa)*t.
"""

import time

import numpy as np
import ml_dtypes

import jax
from jax.sharding import Mesh, PartitionSpec, NamedSharding
from jax.experimental.shard_map import shard_map

import concourse.bacc as bacc
import concourse.mybir as mybir
import concourse.tile as tile
from concourse.bass2jax import (
    _bass_exec_p,
    install_neuronx_cc_hook,
    partition_id_tensor,
)

F32 = mybir.dt.float32
BF16 = mybir.dt.bfloat16
ALU = mybir.AluOpType
AXL = mybir.AxisListType
ACTF = mybir.ActivationFunctionType

N_CORES = 8
B_FULL = 32
BC = B_FULL // N_CORES  # samples per core

C_LIST = [8, 16, 24, 32, 48]
HW = [256, 128, 64, 32, 16]
C_OFF = [0, 8, 24, 48, 80]  # channel-concat offsets (sum C = 128)

NSB = [1, 1, 2, 4, 4]       # samples packed per SBUF tile
NTAU = [2, 1, 1, 1, 1]      # h-tiles per sample (scale 0: 256 rows -> 2)
PARTS = [128, 128, 128, 128, 64]
CP = [4, 8, 8, 32, 48]      # channels per final-pass chunk
PP_BASE = [0, 32, 96, 144, 176]  # pooled-psum col bases

LAST_EXEC_NS = None
LAST_COMPILE_S = None


def _tiles(i):
    """Tile descriptors (s_list, h0) for scale i."""
    out = []
    if i == 0:
        for s in range(BC):
            for tau in range(2):
                out.append(([s], tau * 128))
    elif i == 1:
        for s in range(BC):
            out.append(([s], 0))
    elif i == 2:
        out.append(([0, 1], 0))
        out.append(([2, 3], 0))
    else:
        out.append(([0, 1, 2, 3], 0))
    return out


def _conv_structure(i):
    """(terms, matkeys): terms[out_tile] = [(mat_idx, in_tile), ...] in issue
    order; matkeys[m] = (ch, j, tau_in, tau_out)."""
    matkeys = []

    def mk(key):
        if key not in matkeys:
            matkeys.append(key)
        return matkeys.index(key)

    terms = {}
    if i == 0:
        for s in range(BC):
            for tau_o in range(2):
                lst = []
                for ch in range(2):
                    for j in range(7):
                        for tau_i in range(2):
                            # keep only pairs with any in-range tap
                            nz = False
                            for it in range(7):
                                dk = 3 * (it - 3)
                                for ho in range(128):
                                    gi = tau_o * 128 + ho + dk
                                    if 0 <= gi < 256 and tau_i * 128 <= gi < tau_i * 128 + 128:
                                        nz = True
                                        break
                                if nz:
                                    break
                            if nz:
                                lst.append((mk((ch, j, tau_i, tau_o)), s * 2 + tau_i))
                terms[s * 2 + tau_o] = lst
    else:
        for ti in range(len(_tiles(i))):
            lst = []
            for ch in range(2):
                for j in range(7):
                    lst.append((mk((ch, j, 0, 0)), ti))
            terms[ti] = lst
    return terms, matkeys


def _build_band(k7, Cdiv, Hb, nsb, h0_in, h0_out, H):
    P_in = nsb * Hb
    B = np.zeros((P_in, P_in), np.float32)
    for it in range(7):
        dk = 3 * (it - 3)
        for ho in range(Hb):
            gi = h0_out + ho + dk
            hi = gi - h0_in
            if 0 <= gi < H and 0 <= hi < Hb:
                for q in range(nsb):
                    B[q * Hb + hi, q * Hb + ho] += k7[it] / Cdiv
    return B


def _host_consts(sp_w, c1d_w, ws, bs):
    consts = {}
    for i in range(5):
        C, H, nsb, P = C_LIST[i], HW[i], NSB[i], PARTS[i]
        Hb = H if NTAU[i] == 1 else 128
        _, matkeys = _conv_structure(i)
        mats = np.zeros((len(matkeys), P, P), np.float32)
        for m, (ch, j, tau_i, tau_o) in enumerate(matkeys):
            k7 = np.asarray(sp_w)[0, ch, :, j]
            mats[m] = _build_band(k7, C if ch == 0 else 1.0, Hb, nsb,
                                  tau_i * 128, tau_o * 128, H)
        consts[f"B{i+1}"] = mats.astype(ml_dtypes.bfloat16)

    T3 = np.zeros((128, 128), np.float32)
    w3 = np.asarray(c1d_w)[0, 0]
    for m in range(128):
        for tau in range(3):
            k_ = m + tau - 1
            if 0 <= k_ < 128:
                T3[k_, m] = w3[tau]
    consts["T3"] = T3

    wiT = np.zeros((128, 128), np.float32)
    bRep = np.zeros((4, 128), np.float32)
    for i in range(5):
        C, off = C_LIST[i], C_OFF[i]
        wiT[:, off:off + C] = np.asarray(ws[i]).T
        bRep[:, off:off + C] = np.tile(np.asarray(bs[i])[None, :], (4, 1))
    consts["wiT"] = wiT
    consts["bRep"] = bRep

    # ones-block rhs for pooled matmuls: [128, 12], scale i at cols ob_cols[i]
    ob = np.zeros((128, 12), np.float32)
    col = 0
    for i in range(5):
        nsb, H = NSB[i], HW[i]
        Hb = H if NTAU[i] == 1 else 128
        for q in range(nsb):
            ob[q * Hb:(q + 1) * Hb, col + q] = 1.0 / (H * H)
        col += nsb
    consts["Oblk"] = ob

    # Lsel replicate matrices packed into [4, 960]
    lsel = np.zeros((4, 960), np.float32)
    col = 0
    for s in range(4):                      # ("samp", s) at col s*128
        lsel[s, col:col + 128] = 1.0
        col += 128
    for k in range(2):                      # ("t3", k) at 512 + 128k
        for sl in range(2):
            lsel[2 * k + sl, col + sl * 64: col + (sl + 1) * 64] = 1.0
        col += 128
    for s in range(4):                      # ("t4",) at 768
        lsel[s, col + s * 32: col + (s + 1) * 32] = 1.0
    col += 128
    for s in range(4):                      # ("t5",) at 896 (64 wide)
        lsel[s, col + s * 16: col + (s + 1) * 16] = 1.0
    consts["Lsel"] = lsel

    consts["ones"] = np.ones((128, 1), np.float32)
    return consts


OB_COLS = [0, 1, 2, 4, 8]  # col base of scale i in Oblk
LSEL_SAMP = [0, 128, 256, 384]
LSEL_T3 = [512, 640]
LSEL_T4 = 768
LSEL_T5 = 896


def _build_program(sp_b_val):
    nc = bacc.Bacc("TRN2", target_bir_lowering=False, debug=False)

    td, od, Bd = [], [], []
    nmats = []
    for i in range(5):
        C, H, P = C_LIST[i], HW[i], PARTS[i]
        td.append(nc.dram_tensor(f"t{i+1}", [BC, C, H, H], F32, kind="ExternalInput").ap())
        od.append(nc.dram_tensor(f"o{i+1}", [BC, C, H, H], F32, kind="ExternalOutput").ap())
        _, matkeys = _conv_structure(i)
        nmats.append(len(matkeys))
        Bd.append(nc.dram_tensor(f"B{i+1}", [len(matkeys), P, P], BF16, kind="ExternalInput").ap())
    T3d = nc.dram_tensor("T3", [128, 128], F32, kind="ExternalInput").ap()
    wiTd = nc.dram_tensor("wiT", [128, 128], F32, kind="ExternalInput").ap()
    bRepd = nc.dram_tensor("bRep", [4, 128], F32, kind="ExternalInput").ap()
    Oblkd = nc.dram_tensor("Oblk", [128, 12], F32, kind="ExternalInput").ap()
    Lseld = nc.dram_tensor("Lsel", [4, 960], F32, kind="ExternalInput").ap()
    onesd = nc.dram_tensor("ones", [128, 1], F32, kind="ExternalInput").ap()

    def dma_tile(sbuf3d, ap, i, s_list, h0, c0=0, cn=None, store=False):
        """DMA between SBUF tile view [(s,h), c, w] and DRAM, one transfer
        per packed sample (DMA APs are limited to 3 dims)."""
        C, H = C_LIST[i], HW[i]
        if cn is None:
            cn = C
        Hb = min(128, H)
        for q, s in enumerate(s_list):
            v = ap[s, c0:c0 + cn, h0:h0 + Hb, :].transpose([1, 0, 2])
            sb = sbuf3d[q * Hb:(q + 1) * Hb]
            if store:
                nc.sync.dma_start(v, sb)
            else:
                nc.sync.dma_start(sb, v)

    with tile.TileContext(nc) as tc:
        with (
            tc.tile_pool(name="persist", bufs=1) as pers,
            tc.tile_pool(name="apads", bufs=8) as apool,
            tc.tile_pool(name="avgtmp", bufs=2) as avgp,
            tc.tile_pool(name="ubuf", bufs=3) as upool,
            tc.tile_pool(name="qbuf", bufs=2) as qpool,
            tc.tile_pool(name="obuf", bufs=3) as opool,
            tc.tile_pool(name="small", bufs=1) as smallp,
            tc.tile_pool(name="convps", bufs=3, space="PSUM") as cvps,
            tc.tile_pool(name="poolps", bufs=1, space="PSUM") as plps,
            tc.tile_pool(name="smallps", bufs=2, space="PSUM") as smps,
        ):
            # ---- constant loads ----
            Bsb = []
            for i in range(5):
                P = PARTS[i]
                b = pers.tile([P, nmats[i] * P], BF16, tag=f"Bsb{i}", name=f"Bsb{i}")
                nc.sync.dma_start(
                    b[:].rearrange("p (m q) -> p m q", q=P),
                    Bd[i].transpose([1, 0, 2]),
                )
                Bsb.append(b)
            T3sb = pers.tile([128, 128], F32, tag="T3sb", name="T3sb")
            nc.sync.dma_start(T3sb[:], T3d)
            wiTsb = pers.tile([128, 128], F32, tag="wiTsb", name="wiTsb")
            nc.sync.dma_start(wiTsb[:], wiTd)
            bRepsb = pers.tile([4, 128], F32, tag="bRepsb", name="bRepsb")
            nc.sync.dma_start(bRepsb[:], bRepd)
            Oblksb = pers.tile([128, 12], F32, tag="Oblksb", name="Oblksb")
            nc.sync.dma_start(Oblksb[:], Oblkd)
            Lselsb = pers.tile([4, 960], F32, tag="Lselsb", name="Lselsb")
            nc.sync.dma_start(Lselsb[:], Lseld)
            onessb = pers.tile([128, 1], F32, tag="onessb", name="onessb")
            nc.sync.dma_start(onessb[:], onesd)

            # ---- persistent per-tile state ----
            tsb = {}   # (i, ti) -> t tile [P, C*W]
            sasb = {}  # (i, ti) -> sa tile [P, W] f32
            uacc = {}  # (i, ti) -> [P, C] f32
            all_tiles = {i: _tiles(i) for i in range(5)}

            # ---- per scale: load, stats, conv, sigmoid, u-pass, pooled ----
            pooledps = plps.tile([4, 224], F32, name="pooledps")
            for i in range(5):
                C, H, W, P, nsb = C_LIST[i], HW[i], HW[i], PARTS[i], NSB[i]
                tiles = all_tiles[i]
                terms, matkeys = _conv_structure(i)

                # t loads
                for ti, (s_list, h0) in enumerate(tiles):
                    t = pers.tile([P, C * W], F32, tag=f"t{i}_{ti}", name=f"t{i}_{ti}")
                    dma_tile(t[:].rearrange("p (c w) -> p c w", w=W),
                             td[i], i, s_list, h0)
                    tsb[(i, ti)] = t

                # stats -> padded bf16 a-maps
                apad = {}
                for ti in range(len(tiles)):
                    t = tsb[(i, ti)]
                    tv = t[:].rearrange("p (c w) -> p w c", w=W)
                    aa = apool.tile([P, W + 18], BF16, tag="apad", name=f"aavg{i}_{ti}")
                    am = apool.tile([P, W + 18], BF16, tag="apad", name=f"amax{i}_{ti}")
                    nc.vector.memset(aa[:], 0.0)
                    nc.vector.memset(am[:], 0.0)
                    at = avgp.tile([128, 256], F32, tag="avgtmp", name=f"avgtmp{i}_{ti}")
                    nc.vector.tensor_reduce(at[:P, :W], tv, axis=AXL.X, op=ALU.add)
                    nc.scalar.copy(aa[:, 9:9 + W], at[:P, :W])
                    nc.vector.tensor_reduce(am[:, 9:9 + W], tv, axis=AXL.X, op=ALU.max)
                    apad[ti] = (aa, am)

                # conv (banded matmuls) + sigmoid
                for ot in sorted(terms.keys()):
                    lst = terms[ot]
                    ps = cvps.tile([128, 256], F32, tag="convps", name=f"cps{i}_{ot}")
                    for idx, (m, tin) in enumerate(lst):
                        ch, j, _, _ = matkeys[m]
                        amap = apad[tin][ch]
                        nc.tensor.matmul(
                            ps[:P, :W],
                            lhsT=Bsb[i][:, m * P:(m + 1) * P],
                            rhs=amap[:, 3 * j:3 * j + W],
                            start=(idx == 0),
                            stop=(idx == len(lst) - 1),
                        )
                    sa = pers.tile([P, W], F32, tag=f"sa{i}_{ot}", name=f"sa{i}_{ot}")
                    nc.scalar.activation(sa[:], ps[:P, :W], ACTF.Sigmoid,
                                         bias=float(sp_b_val), scale=1.0)
                    sasb[(i, ot)] = sa

                # u-pass: per-channel stt with fused row-sum accum
                for ti in range(len(tiles)):
                    t = tsb[(i, ti)]
                    tv3 = t[:].rearrange("p (c w) -> p c w", w=W)
                    sa = sasb[(i, ti)]
                    ua = pers.tile([P, C], F32, tag=f"ua{i}_{ti}", name=f"ua{i}_{ti}")
                    uacc[(i, ti)] = ua
                    for c in range(C):
                        ub = upool.tile([128, 256], F32, tag="ubuf", name=f"ub{i}_{ti}_{c}")
                        nc.vector.scalar_tensor_tensor(
                            out=ub[:P, :W],
                            in0=sa[:],
                            scalar=1.0,
                            in1=tv3[:, c, :],
                            op0=ALU.add,
                            op1=ALU.mult,
                            accum_out=ua[:, c:c + 1],
                        )

                # pooled partial matmuls -> pooledps regions
                base = PP_BASE[i]
                for ti, (s_list, h0) in enumerate(tiles):
                    ua = uacc[(i, ti)]
                    if i == 0:
                        s = s_list[0]
                        tau = h0 // 128
                        nc.tensor.matmul(
                            pooledps[0:1, s * 8:s * 8 + 8],
                            lhsT=Oblksb[:P, OB_COLS[i]:OB_COLS[i] + 1],
                            rhs=ua[:],
                            start=(tau == 0), stop=(tau == 1),
                        )
                    else:
                        colbase = base + ti * C
                        nc.tensor.matmul(
                            pooledps[0:nsb, colbase:colbase + C],
                            lhsT=Oblksb[:P, OB_COLS[i]:OB_COLS[i] + nsb],
                            rhs=ua[:],
                            start=True, stop=True,
                        )

            # ---- pooled -> pooledCol [128, 4] ----
            pooledSB = smallp.tile([4, 224], F32, tag="pooledSB", name="pooledSB")
            nc.scalar.copy(pooledSB[0:1, 0:96], pooledps[0:1, 0:96])
            nc.scalar.copy(pooledSB[0:2, 96:144], pooledps[0:2, 96:144])
            nc.scalar.copy(pooledSB[0:4, 144:224], pooledps[0:4, 144:224])

            pooledCol = smallp.tile([128, 4], F32, tag="pooledCol", name="pooledCol")
            for s in range(4):
                nc.sync.dma_start(pooledCol[0:8, s:s + 1], pooledSB[0:1, s * 8:s * 8 + 8])
                nc.sync.dma_start(pooledCol[8:24, s:s + 1], pooledSB[0:1, 32 + s * 16:32 + s * 16 + 16])
                k, sl = divmod(s, 2)
                nc.sync.dma_start(pooledCol[24:48, s:s + 1], pooledSB[sl:sl + 1, 96 + k * 24:96 + k * 24 + 24])
                nc.sync.dma_start(pooledCol[48:80, s:s + 1], pooledSB[s:s + 1, 144:176])
                nc.sync.dma_start(pooledCol[80:128, s:s + 1], pooledSB[s:s + 1, 176:224])

            attps = smps.tile([128, 4], F32, tag="sps", name="attps")
            nc.tensor.matmul(attps[:, :], lhsT=T3sb[:], rhs=pooledCol[:], start=True, stop=True)
            attSB = smallp.tile([128, 4], F32, tag="attSB", name="attSB")
            nc.scalar.copy(attSB[:], attps[:])

            # ---- catt chain + cattb1 per tile ----
            cattb = {}
            for i in range(5):
                C, off, P = C_LIST[i], C_OFF[i], PARTS[i]
                cps = smps.tile([4, 48], F32, tag="sps", name=f"cattps{i}")
                nc.tensor.matmul(cps[0:4, 0:C], lhsT=attSB[:], rhs=wiTsb[:, off:off + C],
                                 start=True, stop=True)
                cs1 = smallp.tile([4, 48], F32, tag="cs1", name=f"cs1_{i}")
                nc.vector.tensor_add(cs1[0:4, 0:C], cps[0:4, 0:C], bRepsb[:, off:off + C])
                cs2 = smallp.tile([4, 48], F32, tag="cs2", name=f"cs2_{i}")
                nc.scalar.activation(cs2[0:4, 0:C], cs1[0:4, 0:C], ACTF.Sigmoid)
                cs3 = smallp.tile([4, 48], F32, tag="cs3", name=f"cs3_{i}")
                nc.scalar.add(cs3[0:4, 0:C], cs2[0:4, 0:C], 1.0)
                for ti, (s_list, h0) in enumerate(all_tiles[i]):
                    if i == 0:
                        lcol = LSEL_SAMP[s_list[0]]
                    elif i == 1:
                        lcol = LSEL_SAMP[s_list[0]]
                    elif i == 2:
                        lcol = LSEL_T3[ti]
                    elif i == 3:
                        lcol = LSEL_T4
                    else:
                        lcol = LSEL_T5
                    key = (i, ti)
                    if i == 0 and h0 != 0:
                        cattb[key] = cattb[(i, ti - 1)]
                        continue
                    cbp = smps.tile([128, 48], F32, tag="sps", name=f"cbp{i}_{ti}")
                    nc.tensor.matmul(cbp[0:P, 0:C], lhsT=Lselsb[:, lcol:lcol + P],
                                     rhs=cs3[0:4, 0:C], start=True, stop=True)
                    cb = pers.tile([P, C], F32, tag=f"cb{i}_{ti}", name=f"cb{i}_{ti}")
                    nc.scalar.copy(cb[:], cbp[0:P, 0:C])
                    cattb[key] = cb

            # ---- final: out = ((1+sa)*(1+catt) - 1) * t ----
            for i in range(5):
                C, W, P, cp = C_LIST[i], HW[i], PARTS[i], CP[i]
                for ti, (s_list, h0) in enumerate(all_tiles[i]):
                    t = tsb[(i, ti)]
                    tv3 = t[:].rearrange("p (c w) -> p c w", w=W)
                    sa = sasb[(i, ti)]
                    cb = cattb[(i, ti)]
                    for c0 in range(0, C, cp):
                        q = qpool.tile([128, 1024], F32, tag="qbuf", name=f"q{i}_{ti}_{c0}")
                        q3 = q[:P, :cp * W].rearrange("p (c w) -> p c w", w=W)
                        sav = sa[:, None, :].broadcast_to([P, cp, W])
                        cbv = cb[:, c0:c0 + cp, None].broadcast_to([P, cp, W])
                        nc.vector.scalar_tensor_tensor(
                            out=q3, in0=sav, scalar=1.0, in1=cbv,
                            op0=ALU.add, op1=ALU.mult,
                        )
                        ob = opool.tile([128, 1024], F32, tag="obuf", name=f"ob{i}_{ti}_{c0}")
                        ob3 = ob[:P, :cp * W].rearrange("p (c w) -> p c w", w=W)
                        nc.vector.scalar_tensor_tensor(
                            out=ob3, in0=q3, scalar=-1.0, in1=tv3[:, c0:c0 + cp, :],
                            op0=ALU.add, op1=ALU.mult,
                        )
                        dma_tile(ob3, od[i], i, s_list, h0, c0, cp, store=True)

    nc.compile()
    return nc


def _run_spmd(nc, in_maps, n_reps_timed=3):
    """Replicates bass2jax.run_bass_via_pjrt multi-core path, with timing."""
    global LAST_EXEC_NS
    install_neuronx_cc_hook()

    partition_name = nc.partition_id_tensor.name if nc.partition_id_tensor else None
    in_names, out_names, out_avals, zero_outs = [], [], [], []
    for alloc in nc.m.functions[0].allocations:
        if not isinstance(alloc, mybir.MemoryLocationSet):
            continue
        name = alloc.memorylocations[0].name
        if alloc.kind == "ExternalInput":
            if name != partition_name:
                in_names.append(name)
        elif alloc.kind == "ExternalOutput":
            shape = tuple(alloc.tensor_shape)
            dtype = mybir.dt.np(alloc.dtype)
            out_names.append(name)
            out_avals.append(jax.core.ShapedArray(shape, dtype))
            zero_outs.append(np.zeros(shape, dtype))
    n_params = len(in_names)
    n_outs = len(out_avals)
    in_names.extend(out_names)
    if partition_name is not None:
        in_names.append(partition_name)

    def _per_core_inputs(in_map):
        return [np.asarray(in_map[name]) for name in in_names[:n_params]]

    donate = tuple(range(n_params, n_params + n_outs))

    def _body(*args):
        operands = list(args)
        if partition_name is not None:
            operands.append(partition_id_tensor())
        outs = _bass_exec_p.bind(
            *operands,
            out_avals=tuple(out_avals),
            in_names=tuple(in_names),
            out_names=tuple(out_names),
            lowering_input_output_aliases=(),
            sim_require_finite=True,
            sim_require_nnan=True,
            nc=nc,
        )
        return tuple(outs)

    n_cores = len(in_maps)
    devices = jax.devices()[:n_cores]
    mesh = Mesh(np.asarray(devices), ("core",))
    in_specs = (PartitionSpec("core"),) * (n_params + n_outs)
    out_specs = (PartitionSpec("core"),) * len(out_names)
    sharded = jax.jit(
        shard_map(_body, mesh=mesh, in_specs=in_specs, out_specs=out_specs,
                  check_rep=False),
        donate_argnums=donate, keep_unused=True,
    )
    per_core = [_per_core_inputs(m) for m in in_maps]
    concat_in = [
        np.concatenate([per_core[c][i] for c in range(n_cores)], axis=0)
        for i in range(n_params)
    ]

    def fresh_zeros():
        return [np.zeros((n_cores * z.shape[0], *z.shape[1:]), z.dtype)
                for z in zero_outs]

    out_arrs = sharded(*concat_in, *fresh_zeros())
    jax.block_until_ready(out_arrs)
    results = [
        {name: np.asarray(out_arrs[i]).reshape(n_cores, *out_avals[i].shape)[c]
         for i, name in enumerate(out_names)}
        for c in range(n_cores)
    ]

    # timed re-runs with device-resident inputs
    try:
        sh = NamedSharding(mesh, PartitionSpec("core"))
        dev_in = [jax.device_put(x, sh) for x in concat_in]
        jax.block_until_ready(dev_in)
        best = None
        for _ in range(max(1, n_reps_timed)):
            zs = [jax.device_put(z, sh) for z in fresh_zeros()]
            jax.block_until_ready(zs)
            t0 = time.perf_counter()
            oo = sharded(*dev_in, *zs)
            jax.block_until_ready(oo)
            dt = time.perf_counter() - t0
            best = dt if best is None else min(best, dt)
        LAST_EXEC_NS = int(best * 1e9)
    except Exception:
        LAST_EXEC_NS = None
    return results


def kernel(t1, t2, t3, t4, t5, sp_w, sp_b, c1d_w,
           w1, b1, w2, b2, w3, b3, w4, b4, w5, b5):
    global LAST_COMPILE_S
    ts = [np.asarray(t1), np.asarray(t2), np.asarray(t3), np.asarray(t4), np.asarray(t5)]
    ws = [np.asarray(w) for w in (w1, w2, w3, w4, w5)]
    bs = [np.asarray(b) for b in (b1, b2, b3, b4, b5)]

    consts = _host_consts(np.asarray(sp_w), np.asarray(c1d_w), ws, bs)
    c0 = time.perf_counter()
    nc = _build_program(float(np.asarray(sp_b).reshape(-1)[0]))
    LAST_COMPILE_S = time.perf_counter() - c0

    in_maps = []
    for core in range(N_CORES):
        m = {}
        for i in range(5):
            m[f"t{i+1}"] = np.ascontiguousarray(ts[i][core * BC:(core + 1) * BC])
        for k, v in consts.items():
            m[k] = v
        in_maps.append(m)

    results = _run_spmd(nc, in_maps)
    outs = []
    for i in range(5):
        outs.append(np.concatenate([results[c][f"o{i+1}"] for c in range(N_CORES)], axis=0))
    return tuple(outs)
